# revision 20
# baseline (speedup 1.0000x reference)
"""Trainium2 Bass kernel for nn_Attention_4380866642117.

Math: the reference computes additive-score attention
    score[b,i,j] = q[b,i].w_q + k[b,j].w_k ; masked (mask==True -> -1e10)
    attn = softmax_j(score); out = LN(attn @ v @ fc_w.T + q)
Because the score is additive, the q-term is constant along the softmax axis
and cancels; masked logits (-1e10) underflow to exactly 0 in the f32 softmax.
Hence with e_j = exp(k[b,j].w_k):
    attn[b,i,j] = (1-m[b,i,j]) * e_j / Z_i,  Z_i = sum_j (1-m) e_j
    out_pre_ln[b,i,:] = (1/Z_i) * sum_j (1-m) e_j (v[b,j,:] @ fc_w.T) + q
so the whole attention+fc collapses to one masked matmul with fc-transformed
values, plus a rank-1 elementwise product for the attn output.

Distribution: data-parallel over batch (8 batches -> 8 NeuronCores).

The mask must enter the PE with j on partitions (contraction dim).  The u8
mask is loaded a second time through the DMA xbar transpose by viewing pairs
of mask bytes as one fp16 element: the transposed SBUF tile then holds rows
j=2p and j=2p+1 interleaved on partition p.  The resulting j-permutation
sigma(c,p,b) = 256c + 2p + b is absorbed by loading k and v rows with the
same permuted access pattern; the contraction is order-invariant.
"""

import numpy as np

import concourse.bass as bass
import concourse.tile as tile
from concourse import mybir
from concourse.bass_utils import run_bass_kernel_spmd

F32 = mybir.dt.float32
F16 = mybir.dt.float16
U8 = mybir.dt.uint8
AF = mybir.ActivationFunctionType
ALU = mybir.AluOpType

B, LQ, LK, D = 8, 2048, 2048, 256
DEBUG_OUTPUTS = False
NT = LK // 128  # 16 j-tiles (and i-tiles)
NG = 4  # i-tile groups
GS = NT // NG  # i-tiles per group
LN_EPS = 1e-5

# ---------------------------------------------------------------------------
# walrus in this container supports at most ONE sync wait per instruction;
# Tile emits several.  Hoist excess waits onto same-engine NoOps.
# ---------------------------------------------------------------------------


def _fix_sync_waits(nc, max_waits: int = 1) -> int:
    n = 0
    for f in nc.m.functions:
        for bb in f.blocks:
            out = []
            changed = False
            for inst in bb.instructions:
                si = inst.sync_info
                waits = list(si.on_wait) if si is not None else []
                if len(waits) > max_waits and inst.engine is not None:
                    changed = True
                    rest = waits[max_waits:]
                    for i in range(0, len(rest), max_waits):
                        nop = mybir.InstNoOp(
                            name=f"{inst.name}-syncw{n}",
                            sync_info=mybir.SyncInfo(
                                on_wait=rest[i : i + max_waits], on_update=[]
                            ),
                            bass_nofuse=True,
                            engine=inst.engine,
                        )
                        n += 1
                        out.append(nop)
                    inst.sync_info = mybir.SyncInfo(
                        on_wait=waits[:max_waits], on_update=list(si.on_update)
                    )
                out.append(inst)
            if changed:
                bb.instructions = out
    return n


def _bcast_part(ap, parts=128):
    """Broadcast a [1, ...] AP across partitions (step-0 partition dim)."""
    return bass.AP(
        tensor=ap.tensor, offset=ap.offset, ap=[[0, parts]] + list(ap.ap[1:])
    )


def _bcast_mid(ap, n):
    """Insert a step-0 middle dim: [p, f] -> [p, n, f]."""
    return bass.AP(
        tensor=ap.tensor,
        offset=ap.offset,
        ap=[list(ap.ap[0]), [0, n]] + [list(a) for a in ap.ap[1:]],
    )


def build_nc():
    nc = bass.Bass("TRN2")

    q_d = nc.dram_tensor("q", [LQ, D], F32, kind="ExternalInput")
    k_d = nc.dram_tensor("k", [LK, D], F32, kind="ExternalInput")
    v_d = nc.dram_tensor("v", [LK, D], F32, kind="ExternalInput")
    mask_d = nc.dram_tensor("mask", [LQ, LK], U8, kind="ExternalInput")
    wk_d = nc.dram_tensor("wk", [1, D], F32, kind="ExternalInput")
    fcw_d = nc.dram_tensor("fcw", [D, D], F32, kind="ExternalInput")
    gamma_d = nc.dram_tensor("gamma", [1, D], F32, kind="ExternalInput")
    beta_d = nc.dram_tensor("beta", [1, D], F32, kind="ExternalInput")
    ident_d = nc.dram_tensor("ident", [128, 128], F32, kind="ExternalInput")
    pb_d = nc.dram_tensor("pb", [128, 2, 256], F32, kind="ExternalInput")

    out_d = nc.dram_tensor("out", [LQ, D], F32, kind="ExternalOutput")
    attn_d = nc.dram_tensor("attn", [LQ, LK], F32, kind="ExternalOutput")

    # permuted row views: j = 256c + 2p + b  ->  tile t = 2c + b, partition p
    k_perm = k_d[:, :].rearrange("(c p two) d -> p c two d", p=128, two=2)
    v_perm = v_d[:, :].rearrange("(c p two) d -> p c two d", p=128, two=2)
    q_grp = q_d[:, :].rearrange("(g ii p) d -> g p ii d", p=128, ii=GS)
    out_grp = out_d[:, :].rearrange("(g ii p) d -> g p ii d", p=128, ii=GS)
    # mask viewed as fp16 pairs for the xbar transpose
    mask_f16 = mask_d[:, :].bitcast(F16)  # [2048, 1024]

    with tile.TileContext(nc) as tc:
        with (
            tc.tile_pool(name="const", bufs=1) as const,
            tc.tile_pool(name="stat", bufs=1) as stat,
            tc.tile_pool(name="kvp", bufs=1) as kvp,
            tc.tile_pool(name="dump", bufs=2) as dumpp,
            tc.tile_pool(name="mt", bufs=2) as mtp,
            tc.tile_pool(name="attnp", bufs=2) as attnp,
            tc.tile_pool(name="mnat", bufs=3) as mnatp,
            tc.tile_pool(name="grp", bufs=2) as grp,
            tc.tile_pool(name="grp1", bufs=1) as grp1,
            tc.tile_pool(name="small", bufs=2) as small,
            tc.tile_pool(name="psm", bufs=2, space="PSUM") as psm,
            tc.tile_pool(name="psB", bufs=2, space="PSUM") as psB,
            tc.tile_pool(name="psN", bufs=4, space="PSUM") as psN,
        ):
            # ---------------- constants ----------------
            ident = const.tile([128, 128], F32)
            nc.sync.dma_start(out=ident, in_=ident_d[:, :])
            pb = const.tile([128, 2, 256], F32)
            nc.sync.dma_start(out=pb, in_=pb_d[:, :, :])
            wk_bc = const.tile([128, D], F32)
            nc.gpsimd.dma_start(out=wk_bc, in_=_bcast_part(wk_d[0:1, :]))
            gamma_bc = const.tile([128, D], F32)
            nc.gpsimd.dma_start(out=gamma_bc, in_=_bcast_part(gamma_d[0:1, :]))
            beta_bc = const.tile([128, D], F32)
            nc.gpsimd.dma_start(out=beta_bc, in_=_bcast_part(beta_d[0:1, :]))
            ones_row = const.tile([1, 128], F32)
            nc.vector.memset(ones_row, 1.0)
            eps_sb = const.tile([128, 1], F32)
            nc.vector.memset(eps_sb, LN_EPS)

            # ---------------- persistent activations ----------------
            sk_sb = stat.tile([128, NT], F32)
            e_perm = stat.tile([128, NT], F32)
            neg_e = stat.tile([128, NT], F32)
            zinv = stat.tile([128, NT], F32)
            ebcast = stat.tile([128, LK], F32)
            tT = stat.tile([128, NT, LQ], F16)
            evW1 = stat.tile([128, NT, 257], F16)

            # ---------------- big input loads ----------------
            k_sb = kvp.tile([128, 8, 2, D], F32, tag="k")
            nc.sync.dma_start(out=k_sb, in_=k_perm)
            v_sb = kvp.tile([128, 8, 2, D], F32, tag="v")
            nc.sync.dma_start(out=v_sb, in_=v_perm)

            # fcwT[d, c] tiles from fcw[c, d]
            fcw_sb = const.tile([128, 2, 256], F32)
            nc.sync.dma_start(
                out=fcw_sb, in_=fcw_d[:, :].rearrange("(t p) d -> p t d", p=128)
            )

            # transposed mask loads: grouped together (the xbar serializes
            # against all other DMA traffic), right after k/v/fcw.  Split in
            # i-halves so the first groups' matmuls/drains start while the
            # second half still transposes.
            mt_tiles = {}
            for h in range(2):
                for c in range(8):
                    mt = mtp.tile(
                        [128, LQ // 2], F16, tag="mt", name=f"mt_{h}_{c}"
                    )
                    nc.sync.dma_start_transpose(
                        mt,
                        mask_f16[
                            1024 * h : 1024 * (h + 1), 128 * c : 128 * (c + 1)
                        ],
                    )
                    mt_tiles[(h, c)] = mt

            # natural mask prefetch (rolling) on the scalar ring (idle early)
            mnat_tiles = []
            for it in range(NT):
                mnat = mnatp.tile([128, LK], U8, tag="mnat", name=f"mnat_{it}")
                nc.scalar.dma_start(
                    out=mnat, in_=mask_d[128 * it : 128 * (it + 1), :]
                )
                mnat_tiles.append(mnat)
            fcwT = const.tile([128, 2, 256], F32)
            for dt in range(2):
                ps_t = psm.tile([128, 512], F32, tag="ps")
                for ct in range(2):
                    nc.tensor.transpose(
                        ps_t[:, 128 * ct : 128 * (ct + 1)],
                        fcw_sb[:, ct, 128 * dt : 128 * (dt + 1)],
                        ident,
                    )
                nc.vector.tensor_copy(fcwT[:, dt, :], ps_t[:, 0:256])

            # ---------------- stage A: sk = k @ w_k (permuted rows) -------
            for t in range(NT):
                dump = dumpp.tile([128, D], F32)
                nc.vector.scalar_tensor_tensor(
                    out=dump,
                    in0=k_sb[:, t // 2, t % 2, :],
                    scalar=1.0,
                    in1=wk_bc,
                    op0=ALU.bypass,
                    op1=ALU.mult,
                    accum_out=sk_sb[:, t : t + 1],
                )
            nc.scalar.activation(out=e_perm, in_=sk_sb, func=AF.Exp)
            nc.vector.tensor_scalar_mul(neg_e, e_perm, -1.0)

            # ---------------- stage B: tT = (1 - m^T)  (fp16, exact) ------
            # pure convert, independent of k/e; ACT takes c0-5, DVE c6-7
            def build_tT(h, c):
                mt_u8 = mt_tiles[(h, c)][:].bitcast(U8)  # [128, LQ]
                sl = slice(1024 * h, 1024 * (h + 1))
                for b in range(2):
                    t = 2 * c + b
                    if c < 6:
                        nc.scalar.activation(
                            out=tT[:, t, sl],
                            in_=mt_u8[:, b::2],
                            func=AF.Identity,
                            bias=1.0,
                            scale=-1.0,
                        )
                    else:
                        nc.vector.tensor_scalar(
                            out=tT[:, t, sl],
                            in0=mt_u8[:, b::2],
                            scalar1=-1.0,
                            scalar2=1.0,
                            op0=ALU.mult,
                            op1=ALU.add,
                        )

            for c in range(6):
                build_tT(0, c)

            # ---------------- stage E: e_nat row + ebcast ----------------
            for ch in range(4):
                ps_en = psm.tile([1, 512], F32, tag="ps")
                for cc in range(2):
                    c = 2 * ch + cc
                    for b in range(2):
                        nc.tensor.matmul(
                            ps_en[0:1, 256 * cc : 256 * (cc + 1)],
                            lhsT=e_perm[:, 2 * c + b : 2 * c + b + 1],
                            rhs=pb[:, b, :],
                            start=(b == 0),
                            stop=(b == 1),
                        )
                e_nat_c = small.tile([1, 512], F32, tag="enat", name=f"en_{ch}")
                nc.scalar.copy(e_nat_c, ps_en)
                ps_eb = psm.tile([128, 512], F32, tag="ps")
                nc.tensor.matmul(
                    ps_eb, lhsT=ones_row, rhs=e_nat_c, start=True, stop=True
                )
                nc.vector.tensor_copy(ebcast[:, 512 * ch : 512 * (ch + 1)], ps_eb)

            # ---------------- stage C: evW1 = [(v@fcw.T) | 1] ------------
            for t in range(NT):
                v_tile = v_sb[:, t // 2, t % 2, :]
                ps_vt = psm.tile([128, 512], F32, tag="ps")
                for dh in range(2):
                    nc.tensor.transpose(
                        ps_vt[:, 128 * dh : 128 * (dh + 1)],
                        v_tile[:, 128 * dh : 128 * (dh + 1)],
                        ident,
                    )
                vT_sb = dumpp.tile([128, D], F32, tag="vT")
                nc.scalar.copy(vT_sb, ps_vt[:, 0:256])
                ps_vw = psB.tile([128, 256], F32)
                for dt in range(2):
                    nc.tensor.matmul(
                        ps_vw,
                        lhsT=vT_sb[:, 128 * dt : 128 * (dt + 1)],
                        rhs=fcwT[:, dt, :],
                        start=(dt == 0),
                        stop=(dt == 1),
                    )
                nc.vector.tensor_scalar_mul(
                    evW1[:, t, 0:256], ps_vw, e_perm[:, t : t + 1]
                )
                nc.vector.tensor_copy(evW1[:, t, 256:257], e_perm[:, t : t + 1])

            # DVE-side tT builds last (their transposes arrive latest)
            for c in range(6, 8):
                build_tT(0, c)
            for c in range(8):
                build_tT(1, c)

            # ---------------- stage D: t-outer accumulation --------------
            for g in range(NG):
                ps_tiles = [
                    psN.tile([128, 257], F32, tag="num", name=f"num_{g}_{i_}")
                    for i_ in range(GS)
                ]
                for t in range(NT):
                    for ii in range(GS):
                        it = GS * g + ii
                        nc.tensor.matmul(
                            ps_tiles[ii],
                            lhsT=tT[:, t, 128 * it : 128 * (it + 1)],
                            rhs=evW1[:, t, :],
                            start=(t == 0),
                            stop=(t == NT - 1),
                        )

                # ---- group drain ----
                y_g = grp.tile([128, GS, D], F32, tag="y", name=f"y_{g}")
                for ii in range(GS):
                    it = GS * g + ii
                    zcol = zinv[:, it : it + 1]
                    nc.vector.reciprocal(zcol, ps_tiles[ii][:, 256:257])
                    nc.scalar.mul(y_g[:, ii, :], ps_tiles[ii][:, 0:256], zcol)

                # attn tiles: attn = (mask==0) * (e_j * zinv_i), in-place
                for ii in range(GS):
                    it = GS * g + ii
                    zcol = zinv[:, it : it + 1]
                    at = attnp.tile([128, LK], F32, tag="at", name=f"at_{g}_{ii}")
                    if ii == 1:
                        nc.scalar.mul(at, ebcast, zcol)
                    elif ii == 2:
                        nc.gpsimd.tensor_scalar_mul(at, ebcast, zcol)
                    else:
                        nc.vector.tensor_scalar_mul(at, ebcast, zcol)
                    nc.vector.scalar_tensor_tensor(
                        out=at,
                        in0=mnat_tiles[it],
                        scalar=0.0,
                        in1=at,
                        op0=ALU.is_equal,
                        op1=ALU.mult,
                    )
                    nc.scalar.dma_start(
                        out=attn_d[128 * it : 128 * (it + 1), :], in_=at
                    )

                # residual + layernorm, batched over the group
                q_g = grp1.tile([128, GS, D], F32, tag="qg", name=f"q_{g}")
                nc.scalar.dma_start(out=q_g, in_=q_grp[g])
                x2_g = grp1.tile([128, GS, D], F32, tag="x2", name=f"x2_{g}")
                nc.gpsimd.tensor_tensor(x2_g, y_g, q_g, ALU.add)
                stats_g = small.tile([128, GS, 6], F32, tag="stats", name=f"st_{g}")
                for ii in range(GS):
                    nc.vector.bn_stats(stats_g[:, ii, :], x2_g[:, ii, :])
                mv_g = small.tile([128, GS, 2], F32, tag="mv", name=f"mv_{g}")
                for ii in range(GS):
                    nc.vector.bn_aggr(mv_g[:, ii, :], stats_g[:, ii, :])
                rstd_g = small.tile([128, GS], F32, tag="rstd", name=f"rs_{g}")
                nc.scalar.activation(
                    out=rstd_g, in_=mv_g[:, :, 1], func=AF.Sqrt, bias=eps_sb,
                    scale=1.0,
                )
                nc.vector.reciprocal(rstd_g, rstd_g)
                nmr_g = small.tile([128, GS], F32, tag="nmr", name=f"nm_{g}")
                nc.vector.scalar_tensor_tensor(
                    out=nmr_g,
                    in0=mv_g[:, :, 0],
                    scalar=-1.0,
                    in1=rstd_g,
                    op0=ALU.mult,
                    op1=ALU.mult,
                )
                s_g = grp1.tile([128, GS, D], F32, tag="s", name=f"s_{g}")
                for ii in range(GS):
                    nc.scalar.activation(
                        out=s_g[:, ii, :],
                        in_=x2_g[:, ii, :],
                        func=AF.Identity,
                        scale=rstd_g[:, ii : ii + 1],
                        bias=nmr_g[:, ii : ii + 1],
                    )
                sg_g = grp1.tile([128, GS, D], F32, tag="sg", name=f"sg_{g}")
                nc.gpsimd.tensor_tensor(
                    sg_g, s_g, _bcast_mid(gamma_bc[:, :], GS), ALU.mult
                )
                out_g = grp1.tile([128, GS, D], F32, tag="outg", name=f"o_{g}")
                nc.gpsimd.tensor_tensor(
                    out_g, sg_g, _bcast_mid(beta_bc[:, :], GS), ALU.add
                )
                nc.scalar.dma_start(out=out_grp[g], in_=out_g)

    _fix_sync_waits(nc)
    return nc


_NC = None
_LAST_IN_MAPS = None


def _get_nc():
    global _NC
    if _NC is None:
        _NC = build_nc()
    return _NC


def kernel(q, k, v, shared_attn, fc_w, ln_gamma, ln_beta, mask):
    q = np.asarray(q)
    k = np.asarray(k)
    v = np.asarray(v)
    shared_attn = np.asarray(shared_attn)
    fc_w = np.asarray(fc_w)
    ln_gamma = np.asarray(ln_gamma)
    ln_beta = np.asarray(ln_beta)
    mask_u8 = np.asarray(mask).view(np.uint8)

    wk = np.ascontiguousarray(shared_attn[:, D:])  # [1, 256]
    gamma = np.ascontiguousarray(ln_gamma.reshape(1, D)).astype(np.float32)
    beta = np.ascontiguousarray(ln_beta.reshape(1, D)).astype(np.float32)
    ident = np.eye(128, dtype=np.float32)
    # pb[p, b, n] = 1 if n == 2p+b
    pb = np.zeros((128, 2, 256), dtype=np.float32)
    p_idx = np.arange(128)
    for b in range(2):
        pb[p_idx, b, 2 * p_idx + b] = 1.0

    nc = _get_nc()
    in_maps = []
    for b_i in range(B):
        in_maps.append(
            {
                "q": np.ascontiguousarray(q[b_i]),
                "k": np.ascontiguousarray(k[b_i]),
                "v": np.ascontiguousarray(v[b_i]),
                "mask": np.ascontiguousarray(mask_u8[b_i]),
                "wk": wk,
                "fcw": np.ascontiguousarray(fc_w),
                "gamma": gamma,
                "beta": beta,
                "ident": ident,
                "pb": pb,
            }
        )
    global _LAST_IN_MAPS
    _LAST_IN_MAPS = in_maps
    res = run_bass_kernel_spmd(nc, in_maps, core_ids=list(range(B)))
    out = np.stack([res.results[c]["out"] for c in range(B)])
    attn = np.stack([res.results[c]["attn"] for c in range(B)])
    return out, attn


# revision 21
# speedup vs baseline: 1.6171x; 1.6171x over previous
"""Trainium2 Bass kernel for nn_Attention_4380866642117.

Math: the reference computes additive-score attention
    score[b,i,j] = q[b,i].w_q + k[b,j].w_k ; masked (mask==True -> -1e10)
    attn = softmax_j(score); out = LN(attn @ v @ fc_w.T + q)
Because the score is additive, the q-term is constant along the softmax axis
and cancels; masked logits (-1e10) underflow to exactly 0 in the f32 softmax.
Hence with e_j = exp(k[b,j].w_k):
    attn[b,i,j] = (1-m[b,i,j]) * e_j / Z_i,  Z_i = sum_j (1-m) e_j
    out_pre_ln[b,i,:] = (1/Z_i) * sum_j (1-m) e_j (v[b,j,:] @ fc_w.T) + q
so the whole attention+fc collapses to one masked matmul with fc-transformed
values, plus a rank-1 elementwise product for the attn output.

Distribution: data-parallel over batch (8 batches -> 8 NeuronCores).

The mask must enter the PE with j on partitions (contraction dim).  The u8
mask is loaded a second time through the DMA xbar transpose by viewing pairs
of mask bytes as one fp16 element: the transposed SBUF tile then holds rows
j=2p and j=2p+1 interleaved on partition p.  The resulting j-permutation
sigma(c,p,b) = 256c + 2p + b is absorbed by loading k and v rows with the
same permuted access pattern; the contraction is order-invariant.
"""

import numpy as np

import concourse.bass as bass
import concourse.tile as tile
from concourse import mybir
from concourse.bass_utils import run_bass_kernel_spmd

F32 = mybir.dt.float32
F16 = mybir.dt.float16
U8 = mybir.dt.uint8
AF = mybir.ActivationFunctionType
ALU = mybir.AluOpType

B, LQ, LK, D = 8, 2048, 2048, 256
DEBUG_OUTPUTS = False
NT = LK // 128  # 16 j-tiles (and i-tiles)
NG = 4  # i-tile groups
GS = NT // NG  # i-tiles per group
LN_EPS = 1e-5

# ---------------------------------------------------------------------------
# walrus in this container supports at most ONE sync wait per instruction;
# Tile emits several.  Hoist excess waits onto same-engine NoOps.
# ---------------------------------------------------------------------------


def _fix_sync_waits(nc, max_waits: int = 1) -> int:
    n = 0
    for f in nc.m.functions:
        for bb in f.blocks:
            out = []
            changed = False
            for inst in bb.instructions:
                si = inst.sync_info
                waits = list(si.on_wait) if si is not None else []
                if len(waits) > max_waits and inst.engine is not None:
                    changed = True
                    rest = waits[max_waits:]
                    for i in range(0, len(rest), max_waits):
                        nop = mybir.InstNoOp(
                            name=f"{inst.name}-syncw{n}",
                            sync_info=mybir.SyncInfo(
                                on_wait=rest[i : i + max_waits], on_update=[]
                            ),
                            bass_nofuse=True,
                            engine=inst.engine,
                        )
                        n += 1
                        out.append(nop)
                    inst.sync_info = mybir.SyncInfo(
                        on_wait=waits[:max_waits], on_update=list(si.on_update)
                    )
                out.append(inst)
            if changed:
                bb.instructions = out
    return n


def _bcast_part(ap, parts=128):
    """Broadcast a [1, ...] AP across partitions (step-0 partition dim)."""
    return bass.AP(
        tensor=ap.tensor, offset=ap.offset, ap=[[0, parts]] + list(ap.ap[1:])
    )


def _bcast_mid(ap, n):
    """Insert a step-0 middle dim: [p, f] -> [p, n, f]."""
    return bass.AP(
        tensor=ap.tensor,
        offset=ap.offset,
        ap=[list(ap.ap[0]), [0, n]] + [list(a) for a in ap.ap[1:]],
    )


def build_nc():
    nc = bass.Bass("TRN2")

    q_d = nc.dram_tensor("q", [LQ, D], F32, kind="ExternalInput")
    k_d = nc.dram_tensor("k", [LK, D], F32, kind="ExternalInput")
    v_d = nc.dram_tensor("v", [LK, D], F32, kind="ExternalInput")
    mask_d = nc.dram_tensor("mask", [LQ, LK], U8, kind="ExternalInput")
    wk_d = nc.dram_tensor("wk", [1, D], F32, kind="ExternalInput")
    fcw_d = nc.dram_tensor("fcw", [D, D], F32, kind="ExternalInput")
    gamma_d = nc.dram_tensor("gamma", [1, D], F32, kind="ExternalInput")
    beta_d = nc.dram_tensor("beta", [1, D], F32, kind="ExternalInput")
    ident_d = nc.dram_tensor("ident", [128, 128], F32, kind="ExternalInput")
    pb_d = nc.dram_tensor("pb", [128, 2, 256], F32, kind="ExternalInput")

    out_d = nc.dram_tensor("out", [LQ, D], F32, kind="ExternalOutput")
    attn_d = nc.dram_tensor("attn", [LQ, LK], F32, kind="ExternalOutput")

    # permuted row views: j = 256c + 2p + b  ->  tile t = 2c + b, partition p
    k_perm = k_d[:, :].rearrange("(c p two) d -> p c two d", p=128, two=2)
    v_perm = v_d[:, :].rearrange("(c p two) d -> p c two d", p=128, two=2)
    q_grp = q_d[:, :].rearrange("(g ii p) d -> g p ii d", p=128, ii=GS)
    out_grp = out_d[:, :].rearrange("(g ii p) d -> g p ii d", p=128, ii=GS)
    # mask viewed as fp16 pairs for the xbar transpose
    mask_f16 = mask_d[:, :].bitcast(F16)  # [2048, 1024]

    with tile.TileContext(nc) as tc:
        with (
            tc.tile_pool(name="const", bufs=1) as const,
            tc.tile_pool(name="stat", bufs=1) as stat,
            tc.tile_pool(name="kvp", bufs=1) as kvp,
            tc.tile_pool(name="dump", bufs=2) as dumpp,
            tc.tile_pool(name="mt", bufs=2) as mtp,
            tc.tile_pool(name="attnp", bufs=2) as attnp,
            tc.tile_pool(name="mnat", bufs=3) as mnatp,
            tc.tile_pool(name="grp", bufs=2) as grp,
            tc.tile_pool(name="grp1", bufs=1) as grp1,
            tc.tile_pool(name="small", bufs=2) as small,
            tc.tile_pool(name="psm", bufs=2, space="PSUM") as psm,
            tc.tile_pool(name="psB", bufs=2, space="PSUM") as psB,
            tc.tile_pool(name="psN", bufs=4, space="PSUM") as psN,
        ):
            # ---------------- constants ----------------
            ident = const.tile([128, 128], F32)
            nc.sync.dma_start(out=ident, in_=ident_d[:, :])
            pb = const.tile([128, 2, 256], F32)
            nc.sync.dma_start(out=pb, in_=pb_d[:, :, :])
            wk_bc = const.tile([128, D], F32)
            nc.gpsimd.dma_start(out=wk_bc, in_=_bcast_part(wk_d[0:1, :]))
            gamma_bc = const.tile([128, D], F32)
            nc.gpsimd.dma_start(out=gamma_bc, in_=_bcast_part(gamma_d[0:1, :]))
            beta_bc = const.tile([128, D], F32)
            nc.gpsimd.dma_start(out=beta_bc, in_=_bcast_part(beta_d[0:1, :]))
            ones_row = const.tile([1, 128], F32)
            nc.vector.memset(ones_row, 1.0)
            eps_sb = const.tile([128, 1], F32)
            nc.vector.memset(eps_sb, LN_EPS)

            # ---------------- persistent activations ----------------
            sk_sb = stat.tile([128, NT], F32)
            e_perm = stat.tile([128, NT], F32)
            neg_e = stat.tile([128, NT], F32)
            zinv = stat.tile([128, NT], F32)
            ebcast = stat.tile([128, LK], F32)
            tT = stat.tile([128, NT, LQ], F16)
            evW1 = stat.tile([128, NT, 257], F16)

            # ---------------- big input loads ----------------
            k_sb = kvp.tile([128, 8, 2, D], F32, tag="k")
            nc.sync.dma_start(out=k_sb, in_=k_perm)
            v_sb = kvp.tile([128, 8, 2, D], F32, tag="v")
            nc.sync.dma_start(out=v_sb, in_=v_perm)

            # fcwT[d, c] tiles from fcw[c, d]
            fcw_sb = const.tile([128, 2, 256], F32)
            nc.sync.dma_start(
                out=fcw_sb, in_=fcw_d[:, :].rearrange("(t p) d -> p t d", p=128)
            )

            # transposed mask loads: grouped together (the xbar serializes
            # against all other DMA traffic), right after k/v/fcw.  Split in
            # i-halves so the first groups' matmuls/drains start while the
            # second half still transposes.
            mt_tiles = {}
            for h in range(2):
                for c in range(8):
                    mt = mtp.tile(
                        [128, LQ // 2], F16, tag="mt", name=f"mt_{h}_{c}"
                    )
                    nc.sync.dma_start_transpose(
                        mt,
                        mask_f16[
                            1024 * h : 1024 * (h + 1), 128 * c : 128 * (c + 1)
                        ],
                    )
                    mt_tiles[(h, c)] = mt

            # natural mask prefetch (rolling) on the scalar ring (idle early)
            mnat_tiles = []
            for it in range(NT):
                mnat = mnatp.tile([128, LK], U8, tag="mnat", name=f"mnat_{it}")
                nc.sync.dma_start(
                    out=mnat, in_=mask_d[128 * it : 128 * (it + 1), :]
                )
                mnat_tiles.append(mnat)
            fcwT = const.tile([128, 2, 256], F32)
            for dt in range(2):
                ps_t = psm.tile([128, 512], F32, tag="ps")
                for ct in range(2):
                    nc.tensor.transpose(
                        ps_t[:, 128 * ct : 128 * (ct + 1)],
                        fcw_sb[:, ct, 128 * dt : 128 * (dt + 1)],
                        ident,
                    )
                nc.vector.tensor_copy(fcwT[:, dt, :], ps_t[:, 0:256])

            # ---------------- stage A: sk = k @ w_k (permuted rows) -------
            for t in range(NT):
                dump = dumpp.tile([128, D], F32)
                nc.vector.scalar_tensor_tensor(
                    out=dump,
                    in0=k_sb[:, t // 2, t % 2, :],
                    scalar=1.0,
                    in1=wk_bc,
                    op0=ALU.bypass,
                    op1=ALU.mult,
                    accum_out=sk_sb[:, t : t + 1],
                )
            nc.scalar.activation(out=e_perm, in_=sk_sb, func=AF.Exp)
            nc.vector.tensor_scalar_mul(neg_e, e_perm, -1.0)

            # ---------------- stage B: tT = (1 - m^T)  (fp16, exact) ------
            # pure convert, independent of k/e; ACT takes c0-5, DVE c6-7
            def build_tT(h, c):
                mt_u8 = mt_tiles[(h, c)][:].bitcast(U8)  # [128, LQ]
                sl = slice(1024 * h, 1024 * (h + 1))
                for b in range(2):
                    t = 2 * c + b
                    if c < 5:
                        nc.scalar.activation(
                            out=tT[:, t, sl],
                            in_=mt_u8[:, b::2],
                            func=AF.Identity,
                            bias=1.0,
                            scale=-1.0,
                        )
                    else:
                        nc.vector.tensor_scalar(
                            out=tT[:, t, sl],
                            in0=mt_u8[:, b::2],
                            scalar1=-1.0,
                            scalar2=1.0,
                            op0=ALU.mult,
                            op1=ALU.add,
                        )

            for c in range(5):
                build_tT(0, c)

            # ---------------- stage E: e_nat row + ebcast ----------------
            for ch in range(4):
                ps_en = psm.tile([1, 512], F32, tag="ps")
                for cc in range(2):
                    c = 2 * ch + cc
                    for b in range(2):
                        nc.tensor.matmul(
                            ps_en[0:1, 256 * cc : 256 * (cc + 1)],
                            lhsT=e_perm[:, 2 * c + b : 2 * c + b + 1],
                            rhs=pb[:, b, :],
                            start=(b == 0),
                            stop=(b == 1),
                        )
                e_nat_c = small.tile([1, 512], F32, tag="enat", name=f"en_{ch}")
                nc.scalar.copy(e_nat_c, ps_en)
                ps_eb = psm.tile([128, 512], F32, tag="ps")
                nc.tensor.matmul(
                    ps_eb, lhsT=ones_row, rhs=e_nat_c, start=True, stop=True
                )
                nc.vector.tensor_copy(ebcast[:, 512 * ch : 512 * (ch + 1)], ps_eb)

            # ---------------- stage C: evW1 = [(v@fcw.T) | 1] ------------
            for t in range(NT):
                v_tile = v_sb[:, t // 2, t % 2, :]
                ps_vt = psm.tile([128, 512], F32, tag="ps")
                for dh in range(2):
                    nc.tensor.transpose(
                        ps_vt[:, 128 * dh : 128 * (dh + 1)],
                        v_tile[:, 128 * dh : 128 * (dh + 1)],
                        ident,
                    )
                vT_sb = dumpp.tile([128, D], F32, tag="vT")
                nc.scalar.copy(vT_sb, ps_vt[:, 0:256])
                ps_vw = psB.tile([128, 256], F32)
                for dt in range(2):
                    nc.tensor.matmul(
                        ps_vw,
                        lhsT=vT_sb[:, 128 * dt : 128 * (dt + 1)],
                        rhs=fcwT[:, dt, :],
                        start=(dt == 0),
                        stop=(dt == 1),
                    )
                nc.vector.tensor_scalar_mul(
                    evW1[:, t, 0:256], ps_vw, e_perm[:, t : t + 1]
                )
                nc.vector.tensor_copy(evW1[:, t, 256:257], e_perm[:, t : t + 1])

            # DVE-side tT builds last (their transposes arrive latest)
            for c in range(5, 8):
                build_tT(0, c)
            for c in range(5):
                build_tT(1, c)
            for c in range(5, 8):
                build_tT(1, c)

            # ---------------- stage D: t-outer accumulation --------------
            for g in range(NG):
                ps_tiles = [
                    psN.tile([128, 257], F32, tag="num", name=f"num_{g}_{i_}")
                    for i_ in range(GS)
                ]
                for t in range(NT):
                    for ii in range(GS):
                        it = GS * g + ii
                        nc.tensor.matmul(
                            ps_tiles[ii],
                            lhsT=tT[:, t, 128 * it : 128 * (it + 1)],
                            rhs=evW1[:, t, :],
                            start=(t == 0),
                            stop=(t == NT - 1),
                        )

                # ---- group drain ----
                y_g = grp.tile([128, GS, D], F32, tag="y", name=f"y_{g}")
                for ii in range(GS):
                    it = GS * g + ii
                    zcol = zinv[:, it : it + 1]
                    nc.vector.reciprocal(zcol, ps_tiles[ii][:, 256:257])
                    nc.scalar.mul(y_g[:, ii, :], ps_tiles[ii][:, 0:256], zcol)

                # attn tiles: attn = (mask==0) * (e_j * zinv_i), in-place
                for ii in range(GS):
                    it = GS * g + ii
                    zcol = zinv[:, it : it + 1]
                    at = attnp.tile([128, LK], F32, tag="at", name=f"at_{g}_{ii}")
                    if ii == 1:
                        nc.scalar.mul(at, ebcast, zcol)
                    else:
                        nc.vector.tensor_scalar_mul(at, ebcast, zcol)
                    nc.vector.scalar_tensor_tensor(
                        out=at,
                        in0=mnat_tiles[it],
                        scalar=0.0,
                        in1=at,
                        op0=ALU.is_equal,
                        op1=ALU.mult,
                    )
                    nc.scalar.dma_start(
                        out=attn_d[128 * it : 128 * (it + 1), :], in_=at
                    )

                # residual + layernorm, batched over the group
                q_g = grp1.tile([128, GS, D], F32, tag="qg", name=f"q_{g}")
                nc.scalar.dma_start(out=q_g, in_=q_grp[g])
                x2_g = grp1.tile([128, GS, D], F32, tag="x2", name=f"x2_{g}")
                nc.gpsimd.tensor_tensor(x2_g, y_g, q_g, ALU.add)
                stats_g = small.tile([128, GS, 6], F32, tag="stats", name=f"st_{g}")
                for ii in range(GS):
                    nc.vector.bn_stats(stats_g[:, ii, :], x2_g[:, ii, :])
                mv_g = small.tile([128, GS, 2], F32, tag="mv", name=f"mv_{g}")
                for ii in range(GS):
                    nc.vector.bn_aggr(mv_g[:, ii, :], stats_g[:, ii, :])
                rstd_g = small.tile([128, GS], F32, tag="rstd", name=f"rs_{g}")
                nc.scalar.activation(
                    out=rstd_g, in_=mv_g[:, :, 1], func=AF.Sqrt, bias=eps_sb,
                    scale=1.0,
                )
                nc.vector.reciprocal(rstd_g, rstd_g)
                nmr_g = small.tile([128, GS], F32, tag="nmr", name=f"nm_{g}")
                nc.vector.scalar_tensor_tensor(
                    out=nmr_g,
                    in0=mv_g[:, :, 0],
                    scalar=-1.0,
                    in1=rstd_g,
                    op0=ALU.mult,
                    op1=ALU.mult,
                )
                s_g = grp1.tile([128, GS, D], F32, tag="s", name=f"s_{g}")
                for ii in range(GS):
                    nc.scalar.activation(
                        out=s_g[:, ii, :],
                        in_=x2_g[:, ii, :],
                        func=AF.Identity,
                        scale=rstd_g[:, ii : ii + 1],
                        bias=nmr_g[:, ii : ii + 1],
                    )
                sg_g = grp1.tile([128, GS, D], F32, tag="sg", name=f"sg_{g}")
                nc.gpsimd.tensor_tensor(
                    sg_g, s_g, _bcast_mid(gamma_bc[:, :], GS), ALU.mult
                )
                out_g = grp1.tile([128, GS, D], F32, tag="outg", name=f"o_{g}")
                nc.gpsimd.tensor_tensor(
                    out_g, sg_g, _bcast_mid(beta_bc[:, :], GS), ALU.add
                )
                nc.scalar.dma_start(out=out_grp[g], in_=out_g)

    _fix_sync_waits(nc)
    return nc


_NC = None
_LAST_IN_MAPS = None


def _get_nc():
    global _NC
    if _NC is None:
        _NC = build_nc()
    return _NC


def kernel(q, k, v, shared_attn, fc_w, ln_gamma, ln_beta, mask):
    q = np.asarray(q)
    k = np.asarray(k)
    v = np.asarray(v)
    shared_attn = np.asarray(shared_attn)
    fc_w = np.asarray(fc_w)
    ln_gamma = np.asarray(ln_gamma)
    ln_beta = np.asarray(ln_beta)
    mask_u8 = np.asarray(mask).view(np.uint8)

    wk = np.ascontiguousarray(shared_attn[:, D:])  # [1, 256]
    gamma = np.ascontiguousarray(ln_gamma.reshape(1, D)).astype(np.float32)
    beta = np.ascontiguousarray(ln_beta.reshape(1, D)).astype(np.float32)
    ident = np.eye(128, dtype=np.float32)
    # pb[p, b, n] = 1 if n == 2p+b
    pb = np.zeros((128, 2, 256), dtype=np.float32)
    p_idx = np.arange(128)
    for b in range(2):
        pb[p_idx, b, 2 * p_idx + b] = 1.0

    nc = _get_nc()
    in_maps = []
    for b_i in range(B):
        in_maps.append(
            {
                "q": np.ascontiguousarray(q[b_i]),
                "k": np.ascontiguousarray(k[b_i]),
                "v": np.ascontiguousarray(v[b_i]),
                "mask": np.ascontiguousarray(mask_u8[b_i]),
                "wk": wk,
                "fcw": np.ascontiguousarray(fc_w),
                "gamma": gamma,
                "beta": beta,
                "ident": ident,
                "pb": pb,
            }
        )
    global _LAST_IN_MAPS
    _LAST_IN_MAPS = in_maps
    res = run_bass_kernel_spmd(nc, in_maps, core_ids=list(range(B)))
    out = np.stack([res.results[c]["out"] for c in range(B)])
    attn = np.stack([res.results[c]["attn"] for c in range(B)])
    return out, attn


# revision 22
# speedup vs baseline: 1.6361x; 1.0118x over previous
"""Trainium2 Bass kernel for nn_Attention_4380866642117.

Math: the reference computes additive-score attention
    score[b,i,j] = q[b,i].w_q + k[b,j].w_k ; masked (mask==True -> -1e10)
    attn = softmax_j(score); out = LN(attn @ v @ fc_w.T + q)
Because the score is additive, the q-term is constant along the softmax axis
and cancels; masked logits (-1e10) underflow to exactly 0 in the f32 softmax.
Hence with e_j = exp(k[b,j].w_k):
    attn[b,i,j] = (1-m[b,i,j]) * e_j / Z_i,  Z_i = sum_j (1-m) e_j
    out_pre_ln[b,i,:] = (1/Z_i) * sum_j (1-m) e_j (v[b,j,:] @ fc_w.T) + q
so the whole attention+fc collapses to one masked matmul with fc-transformed
values, plus a rank-1 elementwise product for the attn output.

Distribution: data-parallel over batch (8 batches -> 8 NeuronCores).

The mask must enter the PE with j on partitions (contraction dim).  The u8
mask is loaded a second time through the DMA xbar transpose by viewing pairs
of mask bytes as one fp16 element: the transposed SBUF tile then holds rows
j=2p and j=2p+1 interleaved on partition p.  The resulting j-permutation
sigma(c,p,b) = 256c + 2p + b is absorbed by loading k and v rows with the
same permuted access pattern; the contraction is order-invariant.
"""

import numpy as np

import concourse.bass as bass
import concourse.tile as tile
from concourse import mybir
from concourse.bass_utils import run_bass_kernel_spmd

F32 = mybir.dt.float32
F16 = mybir.dt.float16
U8 = mybir.dt.uint8
AF = mybir.ActivationFunctionType
ALU = mybir.AluOpType

B, LQ, LK, D = 8, 2048, 2048, 256
DEBUG_OUTPUTS = False
NT = LK // 128  # 16 j-tiles (and i-tiles)
NG = 4  # i-tile groups
GS = NT // NG  # i-tiles per group
LN_EPS = 1e-5

# ---------------------------------------------------------------------------
# walrus in this container supports at most ONE sync wait per instruction;
# Tile emits several.  Hoist excess waits onto same-engine NoOps.
# ---------------------------------------------------------------------------


def _fix_sync_waits(nc, max_waits: int = 1) -> int:
    n = 0
    for f in nc.m.functions:
        for bb in f.blocks:
            out = []
            changed = False
            for inst in bb.instructions:
                si = inst.sync_info
                waits = list(si.on_wait) if si is not None else []
                if len(waits) > max_waits and inst.engine is not None:
                    changed = True
                    rest = waits[max_waits:]
                    for i in range(0, len(rest), max_waits):
                        nop = mybir.InstNoOp(
                            name=f"{inst.name}-syncw{n}",
                            sync_info=mybir.SyncInfo(
                                on_wait=rest[i : i + max_waits], on_update=[]
                            ),
                            bass_nofuse=True,
                            engine=inst.engine,
                        )
                        n += 1
                        out.append(nop)
                    inst.sync_info = mybir.SyncInfo(
                        on_wait=waits[:max_waits], on_update=list(si.on_update)
                    )
                out.append(inst)
            if changed:
                bb.instructions = out
    return n


def _bcast_part(ap, parts=128):
    """Broadcast a [1, ...] AP across partitions (step-0 partition dim)."""
    return bass.AP(
        tensor=ap.tensor, offset=ap.offset, ap=[[0, parts]] + list(ap.ap[1:])
    )


def _bcast_mid(ap, n):
    """Insert a step-0 middle dim: [p, f] -> [p, n, f]."""
    return bass.AP(
        tensor=ap.tensor,
        offset=ap.offset,
        ap=[list(ap.ap[0]), [0, n]] + [list(a) for a in ap.ap[1:]],
    )


def build_nc():
    nc = bass.Bass("TRN2")

    q_d = nc.dram_tensor("q", [LQ, D], F32, kind="ExternalInput")
    k_d = nc.dram_tensor("k", [LK, D], F32, kind="ExternalInput")
    v_d = nc.dram_tensor("v", [LK, D], F32, kind="ExternalInput")
    mask_d = nc.dram_tensor("mask", [LQ, LK], U8, kind="ExternalInput")
    wk_d = nc.dram_tensor("wk", [1, D], F32, kind="ExternalInput")
    fcw_d = nc.dram_tensor("fcw", [D, D], F32, kind="ExternalInput")
    gamma_d = nc.dram_tensor("gamma", [1, D], F32, kind="ExternalInput")
    beta_d = nc.dram_tensor("beta", [1, D], F32, kind="ExternalInput")
    ident_d = nc.dram_tensor("ident", [128, 128], F32, kind="ExternalInput")
    pb_d = nc.dram_tensor("pb", [128, 2, 256], F32, kind="ExternalInput")

    out_d = nc.dram_tensor("out", [LQ, D], F32, kind="ExternalOutput")
    attn_d = nc.dram_tensor("attn", [LQ, LK], F32, kind="ExternalOutput")

    # permuted row views: j = 256c + 2p + b  ->  tile t = 2c + b, partition p
    k_perm = k_d[:, :].rearrange("(c p two) d -> p c two d", p=128, two=2)
    v_perm = v_d[:, :].rearrange("(c p two) d -> p c two d", p=128, two=2)
    q_grp = q_d[:, :].rearrange("(g ii p) d -> g p ii d", p=128, ii=GS)
    out_grp = out_d[:, :].rearrange("(g ii p) d -> g p ii d", p=128, ii=GS)
    # mask viewed as fp16 pairs for the xbar transpose
    mask_f16 = mask_d[:, :].bitcast(F16)  # [2048, 1024]

    with tile.TileContext(nc) as tc:
        with (
            tc.tile_pool(name="const", bufs=1) as const,
            tc.tile_pool(name="stat", bufs=1) as stat,
            tc.tile_pool(name="kvp", bufs=1) as kvp,
            tc.tile_pool(name="dump", bufs=2) as dumpp,
            tc.tile_pool(name="mt", bufs=2) as mtp,
            tc.tile_pool(name="attnp", bufs=2) as attnp,
            tc.tile_pool(name="mnat", bufs=3) as mnatp,
            tc.tile_pool(name="grp", bufs=2) as grp,
            tc.tile_pool(name="grp1", bufs=1) as grp1,
            tc.tile_pool(name="small", bufs=2) as small,
            tc.tile_pool(name="psm", bufs=2, space="PSUM") as psm,
            tc.tile_pool(name="psB", bufs=2, space="PSUM") as psB,
            tc.tile_pool(name="psN", bufs=4, space="PSUM") as psN,
        ):
            # ---------------- constants (pre-transpose minimum) -----------
            wk_bc = const.tile([128, D], F32)
            nc.gpsimd.dma_start(out=wk_bc, in_=_bcast_part(wk_d[0:1, :]))
            ones_row = const.tile([1, 128], F32)
            nc.vector.memset(ones_row, 1.0)
            eps_sb = const.tile([128, 1], F32)
            nc.vector.memset(eps_sb, LN_EPS)

            # ---------------- persistent activations ----------------
            sk_sb = stat.tile([128, NT], F32)
            e_perm = stat.tile([128, NT], F32)
            neg_e = stat.tile([128, NT], F32)
            zinv = stat.tile([128, NT], F32)
            ebcast = stat.tile([128, LK], F32)
            tT = stat.tile([128, NT, LQ], F16)
            evW1 = stat.tile([128, NT, 257], F16)

            # ---------------- big input loads ----------------
            k_sb = kvp.tile([128, 8, 2, D], F32, tag="k")
            nc.sync.dma_start(out=k_sb, in_=k_perm)
            v_sb = kvp.tile([128, 8, 2, D], F32, tag="v")
            nc.sync.dma_start(out=v_sb, in_=v_perm)

            # fcwT[d, c] tiles from fcw[c, d]
            fcw_sb = const.tile([128, 2, 256], F32)
            nc.sync.dma_start(
                out=fcw_sb, in_=fcw_d[:, :].rearrange("(t p) d -> p t d", p=128)
            )
            ident = const.tile([128, 128], F32)
            nc.sync.dma_start(out=ident, in_=ident_d[:, :])

            # transposed mask loads: grouped together (the xbar serializes
            # against all other DMA traffic), right after k/v/fcw.  Split in
            # i-halves so the first groups' matmuls/drains start while the
            # second half still transposes.
            mt_tiles = {}
            for h in range(2):
                for c in range(8):
                    mt = mtp.tile(
                        [128, LQ // 2], F16, tag="mt", name=f"mt_{h}_{c}"
                    )
                    nc.sync.dma_start_transpose(
                        mt,
                        mask_f16[
                            1024 * h : 1024 * (h + 1), 128 * c : 128 * (c + 1)
                        ],
                    )
                    mt_tiles[(h, c)] = mt

            # post-transpose constants + natural mask prefetch (rolling)
            pb = const.tile([128, 2, 256], F32)
            nc.sync.dma_start(out=pb, in_=pb_d[:, :, :])
            gamma_bc = const.tile([128, D], F32)
            nc.gpsimd.dma_start(out=gamma_bc, in_=_bcast_part(gamma_d[0:1, :]))
            beta_bc = const.tile([128, D], F32)
            nc.gpsimd.dma_start(out=beta_bc, in_=_bcast_part(beta_d[0:1, :]))
            mnat_tiles = []
            for it in range(NT):
                mnat = mnatp.tile([128, LK], U8, tag="mnat", name=f"mnat_{it}")
                nc.sync.dma_start(
                    out=mnat, in_=mask_d[128 * it : 128 * (it + 1), :]
                )
                mnat_tiles.append(mnat)
            fcwT = const.tile([128, 2, 256], F32)
            for dt in range(2):
                ps_t = psm.tile([128, 512], F32, tag="ps")
                for ct in range(2):
                    nc.tensor.transpose(
                        ps_t[:, 128 * ct : 128 * (ct + 1)],
                        fcw_sb[:, ct, 128 * dt : 128 * (dt + 1)],
                        ident,
                    )
                nc.vector.tensor_copy(fcwT[:, dt, :], ps_t[:, 0:256])

            # ---------------- stage A: sk = k @ w_k (permuted rows) -------
            for t in range(NT):
                dump = dumpp.tile([128, D], F32)
                nc.vector.scalar_tensor_tensor(
                    out=dump,
                    in0=k_sb[:, t // 2, t % 2, :],
                    scalar=1.0,
                    in1=wk_bc,
                    op0=ALU.bypass,
                    op1=ALU.mult,
                    accum_out=sk_sb[:, t : t + 1],
                )
            nc.scalar.activation(out=e_perm, in_=sk_sb, func=AF.Exp)
            nc.vector.tensor_scalar_mul(neg_e, e_perm, -1.0)

            # ---------------- stage B: tT = (1 - m^T)  (fp16, exact) ------
            # pure convert, independent of k/e; ACT takes c0-5, DVE c6-7
            def build_tT(h, c):
                mt_u8 = mt_tiles[(h, c)][:].bitcast(U8)  # [128, LQ]
                sl = slice(1024 * h, 1024 * (h + 1))
                for b in range(2):
                    t = 2 * c + b
                    if c < 5:
                        nc.scalar.activation(
                            out=tT[:, t, sl],
                            in_=mt_u8[:, b::2],
                            func=AF.Identity,
                            bias=1.0,
                            scale=-1.0,
                        )
                    else:
                        nc.vector.tensor_scalar(
                            out=tT[:, t, sl],
                            in0=mt_u8[:, b::2],
                            scalar1=-1.0,
                            scalar2=1.0,
                            op0=ALU.mult,
                            op1=ALU.add,
                        )

            for c in range(5):
                build_tT(0, c)

            # ---------------- stage E: e_nat row + ebcast ----------------
            for ch in range(4):
                ps_en = psm.tile([1, 512], F32, tag="ps")
                for cc in range(2):
                    c = 2 * ch + cc
                    for b in range(2):
                        nc.tensor.matmul(
                            ps_en[0:1, 256 * cc : 256 * (cc + 1)],
                            lhsT=e_perm[:, 2 * c + b : 2 * c + b + 1],
                            rhs=pb[:, b, :],
                            start=(b == 0),
                            stop=(b == 1),
                        )
                e_nat_c = small.tile([1, 512], F32, tag="enat", name=f"en_{ch}")
                nc.scalar.copy(e_nat_c, ps_en)
                ps_eb = psm.tile([128, 512], F32, tag="ps")
                nc.tensor.matmul(
                    ps_eb, lhsT=ones_row, rhs=e_nat_c, start=True, stop=True
                )
                nc.vector.tensor_copy(ebcast[:, 512 * ch : 512 * (ch + 1)], ps_eb)

            # ---------------- stage C: evW1 = [(v@fcw.T) | 1] ------------
            for t in range(NT):
                v_tile = v_sb[:, t // 2, t % 2, :]
                ps_vt = psm.tile([128, 512], F32, tag="ps")
                for dh in range(2):
                    nc.tensor.transpose(
                        ps_vt[:, 128 * dh : 128 * (dh + 1)],
                        v_tile[:, 128 * dh : 128 * (dh + 1)],
                        ident,
                    )
                vT_sb = dumpp.tile([128, D], F32, tag="vT")
                nc.scalar.copy(vT_sb, ps_vt[:, 0:256])
                ps_vw = psB.tile([128, 256], F32)
                for dt in range(2):
                    nc.tensor.matmul(
                        ps_vw,
                        lhsT=vT_sb[:, 128 * dt : 128 * (dt + 1)],
                        rhs=fcwT[:, dt, :],
                        start=(dt == 0),
                        stop=(dt == 1),
                    )
                nc.vector.tensor_scalar_mul(
                    evW1[:, t, 0:256], ps_vw, e_perm[:, t : t + 1]
                )
                nc.vector.tensor_copy(evW1[:, t, 256:257], e_perm[:, t : t + 1])

            # DVE-side tT builds last (their transposes arrive latest)
            for c in range(5, 8):
                build_tT(0, c)
            for c in range(5):
                build_tT(1, c)
            for c in range(5, 8):
                build_tT(1, c)

            # ---------------- stage D: t-outer accumulation --------------
            for g in range(NG):
                ps_tiles = [
                    psN.tile([128, 257], F32, tag="num", name=f"num_{g}_{i_}")
                    for i_ in range(GS)
                ]
                for t in range(NT):
                    for ii in range(GS):
                        it = GS * g + ii
                        nc.tensor.matmul(
                            ps_tiles[ii],
                            lhsT=tT[:, t, 128 * it : 128 * (it + 1)],
                            rhs=evW1[:, t, :],
                            start=(t == 0),
                            stop=(t == NT - 1),
                        )

                # ---- group drain ----
                y_g = grp.tile([128, GS, D], F32, tag="y", name=f"y_{g}")
                for ii in range(GS):
                    it = GS * g + ii
                    zcol = zinv[:, it : it + 1]
                    nc.vector.reciprocal(zcol, ps_tiles[ii][:, 256:257])
                    nc.scalar.mul(y_g[:, ii, :], ps_tiles[ii][:, 0:256], zcol)

                # attn tiles: attn = (mask==0) * (e_j * zinv_i), in-place
                for ii in range(GS):
                    it = GS * g + ii
                    zcol = zinv[:, it : it + 1]
                    at = attnp.tile([128, LK], F32, tag="at", name=f"at_{g}_{ii}")
                    if ii == 1:
                        nc.scalar.mul(at, ebcast, zcol)
                    else:
                        nc.vector.tensor_scalar_mul(at, ebcast, zcol)
                    nc.vector.scalar_tensor_tensor(
                        out=at,
                        in0=mnat_tiles[it],
                        scalar=0.0,
                        in1=at,
                        op0=ALU.is_equal,
                        op1=ALU.mult,
                    )
                    if it % 2 == 0:
                        nc.scalar.dma_start(
                            out=attn_d[128 * it : 128 * (it + 1), :], in_=at
                        )
                    else:
                        nc.sync.dma_start(
                            out=attn_d[128 * it : 128 * (it + 1), :], in_=at
                        )

                # residual + layernorm, batched over the group
                q_g = grp1.tile([128, GS, D], F32, tag="qg", name=f"q_{g}")
                nc.scalar.dma_start(out=q_g, in_=q_grp[g])
                x2_g = grp1.tile([128, GS, D], F32, tag="x2", name=f"x2_{g}")
                nc.gpsimd.tensor_tensor(x2_g, y_g, q_g, ALU.add)
                stats_g = small.tile([128, GS, 6], F32, tag="stats", name=f"st_{g}")
                for ii in range(GS):
                    nc.vector.bn_stats(stats_g[:, ii, :], x2_g[:, ii, :])
                mv_g = small.tile([128, GS, 2], F32, tag="mv", name=f"mv_{g}")
                for ii in range(GS):
                    nc.vector.bn_aggr(mv_g[:, ii, :], stats_g[:, ii, :])
                rstd_g = small.tile([128, GS], F32, tag="rstd", name=f"rs_{g}")
                nc.scalar.activation(
                    out=rstd_g, in_=mv_g[:, :, 1], func=AF.Sqrt, bias=eps_sb,
                    scale=1.0,
                )
                nc.vector.reciprocal(rstd_g, rstd_g)
                nmr_g = small.tile([128, GS], F32, tag="nmr", name=f"nm_{g}")
                nc.vector.scalar_tensor_tensor(
                    out=nmr_g,
                    in0=mv_g[:, :, 0],
                    scalar=-1.0,
                    in1=rstd_g,
                    op0=ALU.mult,
                    op1=ALU.mult,
                )
                s_g = grp1.tile([128, GS, D], F32, tag="s", name=f"s_{g}")
                for ii in range(GS):
                    nc.scalar.activation(
                        out=s_g[:, ii, :],
                        in_=x2_g[:, ii, :],
                        func=AF.Identity,
                        scale=rstd_g[:, ii : ii + 1],
                        bias=nmr_g[:, ii : ii + 1],
                    )
                sg_g = grp1.tile([128, GS, D], F32, tag="sg", name=f"sg_{g}")
                nc.gpsimd.tensor_tensor(
                    sg_g, s_g, _bcast_mid(gamma_bc[:, :], GS), ALU.mult
                )
                out_g = grp1.tile([128, GS, D], F32, tag="outg", name=f"o_{g}")
                nc.gpsimd.tensor_tensor(
                    out_g, sg_g, _bcast_mid(beta_bc[:, :], GS), ALU.add
                )
                nc.scalar.dma_start(out=out_grp[g], in_=out_g)

    _fix_sync_waits(nc)
    return nc


_NC = None
_LAST_IN_MAPS = None


def _get_nc():
    global _NC
    if _NC is None:
        _NC = build_nc()
    return _NC


def kernel(q, k, v, shared_attn, fc_w, ln_gamma, ln_beta, mask):
    q = np.asarray(q)
    k = np.asarray(k)
    v = np.asarray(v)
    shared_attn = np.asarray(shared_attn)
    fc_w = np.asarray(fc_w)
    ln_gamma = np.asarray(ln_gamma)
    ln_beta = np.asarray(ln_beta)
    mask_u8 = np.asarray(mask).view(np.uint8)

    wk = np.ascontiguousarray(shared_attn[:, D:])  # [1, 256]
    gamma = np.ascontiguousarray(ln_gamma.reshape(1, D)).astype(np.float32)
    beta = np.ascontiguousarray(ln_beta.reshape(1, D)).astype(np.float32)
    ident = np.eye(128, dtype=np.float32)
    # pb[p, b, n] = 1 if n == 2p+b
    pb = np.zeros((128, 2, 256), dtype=np.float32)
    p_idx = np.arange(128)
    for b in range(2):
        pb[p_idx, b, 2 * p_idx + b] = 1.0

    nc = _get_nc()
    in_maps = []
    for b_i in range(B):
        in_maps.append(
            {
                "q": np.ascontiguousarray(q[b_i]),
                "k": np.ascontiguousarray(k[b_i]),
                "v": np.ascontiguousarray(v[b_i]),
                "mask": np.ascontiguousarray(mask_u8[b_i]),
                "wk": wk,
                "fcw": np.ascontiguousarray(fc_w),
                "gamma": gamma,
                "beta": beta,
                "ident": ident,
                "pb": pb,
            }
        )
    global _LAST_IN_MAPS
    _LAST_IN_MAPS = in_maps
    res = run_bass_kernel_spmd(nc, in_maps, core_ids=list(range(B)))
    out = np.stack([res.results[c]["out"] for c in range(B)])
    attn = np.stack([res.results[c]["attn"] for c in range(B)])
    return out, attn
